# revision 14
# baseline (speedup 1.0000x reference)
"""Distributed contrastive loss kernel for 8 Trainium2 NeuronCores.

loss = mean_i( logsumexp_j(f1n_i . f2n_j / T) - (f1n_i . f2n_i) / T )
with f1n/f2n the L2-row-normalized feature matrices, N=16384, D=512.

v2: fp8 DoubleRow matmuls + chunk-chasing AllGather + wide activations.

- f1 rows sharded 8 ways (2048/core); f2 rows likewise.
- Each core scale-casts its shards to fp8e4 (f1*4; f2*4*inv2 folding the
  row normalization into the quantization scale) and PE-transposes them
  into D-major layout [128, 4, cols].  The local f2 shard is AllGathered
  in 4 column-strip chunks (fp8 halves the wire bytes vs bf16).
- Main loop: per [128, 2048] PSUM tile, 8 DoubleRow fp8 matmuls (2 K-pairs
  x 4 rank-strips of 512 cols) accumulate f32 logits at 2 MACs/cell/cycle.
- Chunk-0 tiles are consumed by ScalarE Exp directly from PSUM (keeps PE
  fed while later AG chunks land); chunks 1-3 are DVE-cast to bf16 SBUF
  and consumed by one wide [128, 12288] Exp per m-tile, all with
  per-partition scale = inv1/(16*T) and accum_out row-sums.
- diag from fp8 PE matmuls of the local shards + identity-mask reduce;
  logsumexp via Ln of accumulated row sums; per-core partial reduced to a
  scalar with a ones-matmul.  Host sums 8 partials / N.
"""

import os
from contextlib import ExitStack
from functools import lru_cache

import numpy as np

import concourse.bass as bass
import concourse.mybir as mybir
import concourse.tile as tile
from concourse.bass_utils import run_bass_kernel_spmd
from concourse.masks import make_identity

# Problem shape (hardcoded per contest rules).
N = 16384
D = 512
N_CORES = 8
M_LOCAL = N // N_CORES  # 2048 rows per core
TEMP = 0.07

P = 128                 # SBUF partitions
DC = D // P             # 4 contraction chunks of 128
KP = DC // 2            # 2 DoubleRow K-pairs (each contracts 256)
MT = M_LOCAL // P       # 16 m-tiles per core
NG = 4                  # AllGather chunks (column strips of 512 per rank)
GT = MT // NG           # 4 f2-shard tiles per chunk
GW = GT * P             # 512 columns per chunk strip
SC = 4.0                # fp8 quantization pre-scale on both operands
F32 = mybir.dt.float32
BF16 = mybir.dt.bfloat16
FP8 = mybir.dt.float8e4
AF = mybir.ActivationFunctionType
ALU = mybir.AluOpType
DR = mybir.MatmulPerfMode.DoubleRow

# Module-level stash for the last run's profile (read by test.py).
LAST_EXEC_TIME_NS = None


def _install_ntff_hook():
    """Provide antenv.axon_hooks (missing from this image) so that
    run_bass_kernel_spmd(trace=True) can capture NTFF profiles via the
    axon PJRT .so. Mirrors trn_agent_boot.trn_boot._ntff_profile_via_ctypes."""
    import contextlib
    import ctypes
    import sys
    import types

    try:
        import antenv.axon_hooks  # noqa: F401

        return
    except ImportError:
        pass

    so_path = "/opt/axon/libaxon_pjrt.so"
    hook = None
    try:
        lib = ctypes.CDLL(so_path)
        if hasattr(lib, "axon_start_nrt_profile"):
            lib.axon_start_nrt_profile.argtypes = [
                ctypes.POINTER(ctypes.c_int64),
                ctypes.c_size_t,
            ]
            lib.axon_start_nrt_profile.restype = ctypes.c_int64
            lib.axon_stop_nrt_profile.argtypes = [ctypes.c_char_p]
            lib.axon_stop_nrt_profile.restype = ctypes.c_int64

            @contextlib.contextmanager
            def _hook(output_dir, device_ids):
                import jax

                jax.devices()
                if device_ids:
                    ids = (ctypes.c_int64 * len(device_ids))(*device_ids)
                    rc = lib.axon_start_nrt_profile(ids, len(device_ids))
                else:
                    rc = lib.axon_start_nrt_profile(None, 0)
                if rc != 0:
                    raise RuntimeError(f"axon_start_nrt_profile rc={rc}")
                try:
                    yield
                finally:
                    n = lib.axon_stop_nrt_profile(str(output_dir).encode())
                    print(f"profile: {n} file(s) written to {output_dir}", file=sys.stderr)

            hook = _hook
    except OSError:
        pass

    import antenv

    mod = types.ModuleType("antenv.axon_hooks")
    _state = {"hook": hook}
    mod.set_axon_ntff_profile_hook = lambda h: _state.__setitem__("hook", h)
    mod.get_axon_ntff_profile_hook = lambda: _state["hook"]
    sys.modules["antenv.axon_hooks"] = mod
    antenv.axon_hooks = mod

    # Artifact upload needs external storage creds; make it a no-op.
    import concourse.bass_utils as _bu

    _bu.upload_artifacts = lambda tmpdir: f"local:{tmpdir}"


def _build_bass():
    nc = bass.Bass(num_devices=N_CORES, debug=False)

    f1s = nc.dram_tensor("f1s", [M_LOCAL, D], F32, kind="ExternalInput")
    f2o = nc.dram_tensor("f2o", [M_LOCAL, D], F32, kind="ExternalInput")
    out = nc.dram_tensor("out", [1, 1], F32, kind="ExternalOutput")

    # Exp scale: psum holds SC^2 * (f1 . f2n); true logit needs * inv1 / T.
    eff_scale = 1.0 / (SC * SC * TEMP)
    WIDE = (NG - 1) * N_CORES * GW  # 12288 cols handled by the wide path

    with tile.TileContext(nc) as tc, ExitStack() as ctx:
        consts = ctx.enter_context(tc.tile_pool(name="consts", bufs=1))
        resident = ctx.enter_context(tc.tile_pool(name="resident", bufs=1))
        loads1 = ctx.enter_context(tc.tile_pool(name="loads1", bufs=4))
        loads2 = ctx.enter_context(tc.tile_pool(name="loads2", bufs=8))
        xq = ctx.enter_context(tc.tile_pool(name="xq", bufs=3))
        stats = ctx.enter_context(tc.tile_pool(name="stats", bufs=4))
        logits = ctx.enter_context(tc.tile_pool(name="logits", bufs=2))
        psum = ctx.enter_context(tc.tile_pool(name="psum", bufs=2, space="PSUM"))
        dram = ctx.enter_context(tc.tile_pool(name="dram", bufs=1, space="DRAM"))

        identity = consts.tile([P, P], BF16)
        make_identity(nc, identity)
        ones_col = consts.tile([P, 1], F32)
        nc.vector.memset(ones_col, 1.0)

        # Residents.  Split per-mtile / per-chunk so the byte-interval dep
        # tracker never sees false write/read overlaps across chunks.
        f1Tt = [
            resident.tile([P, DC, P], FP8, name=f"f1T{t}") for t in range(MT)
        ]
        f2TlocG = [
            resident.tile([P, DC, GW], FP8, name=f"f2Tloc{g}") for g in range(NG)
        ]
        f2Tg = [
            resident.tile([P, DC, N_CORES * GW], FP8, name=f"f2T{g}")
            for g in range(NG)
        ]
        ss1g = resident.tile([P, MT], F32, name="ss1g")
        ss2g = resident.tile([P, MT], F32, name="ss2g")
        inv1g = resident.tile([P, MT], F32, name="inv1g")
        inv2g = resident.tile([P, MT], F32, name="inv2g")
        inv2s = resident.tile([P, MT], F32, name="inv2s")
        scale1g = resident.tile([P, MT], F32, name="scale1g")
        diag_sc2 = resident.tile([P, MT], F32, name="diag_sc2")
        diagterm = resident.tile([P, MT], F32, name="diagterm")
        rs = resident.tile([P, 4 * MT], F32, name="rs")
        losses = resident.tile([P, MT], F32, name="losses")
        gexA = resident.tile([P, N_CORES * GW], FP8, name="gexA")
        gexW = resident.tile([P, WIDE], FP8, name="gexW")
        nc.vector.memset(rs, 0.0)

        # AllGather bounce buffers, one pair per chunk.
        agin = [
            dram.tile([P, DC, GW], FP8, name=f"agin{g}") for g in range(NG)
        ]
        agout = [
            dram.tile([N_CORES, P, DC, GW], FP8, name=f"agout{g}",
                      addr_space="Shared")
            for g in range(NG)
        ]

        def sumsq_col(x, col, tag):
            """sum(x^2) per row of [P, D] f32 tile -> col ([P,1] slice). DVE only."""
            st = stats.tile([P, nc.vector.BN_STATS_DIM], F32, tag="bst", name=f"st{tag}")
            nc.vector.bn_stats(st, x)
            mv = stats.tile([P, nc.vector.BN_AGGR_DIM], F32, tag="bmv", name=f"mv{tag}")
            nc.vector.bn_aggr(mv, st)
            m2 = stats.tile([P, 1], F32, tag="m2", name=f"m2{tag}")
            nc.vector.tensor_tensor(m2, mv[:, 0:1], mv[:, 0:1], ALU.mult)
            s2 = stats.tile([P, 1], F32, tag="s2", name=f"s2{tag}")
            nc.vector.tensor_tensor(s2, mv[:, 1:2], m2, ALU.add)
            nc.vector.tensor_scalar_mul(col, s2, float(D))

        def transpose_pack(xq_tile, dst, dst_col, eng):
            """4x bf16 PE transpose of [P, D] into D-major dst[:, :, dst_col:+P].
            Each 128x128 transpose lands in its own PSUM bank; the fp8
            quantization happens in the PSUM->SBUF copy on `eng`."""
            tp4 = psum.tile([P, DC, 1024], BF16, tag="ps", name="tp4")
            for c in range(DC):
                nc.tensor.matmul(
                    tp4[:, c, 0:P],
                    lhsT=xq_tile[:, c * P : (c + 1) * P],
                    rhs=identity,
                    is_transpose=True,
                )
            if eng is nc.scalar:
                nc.scalar.copy(dst[:, :, dst_col : dst_col + P], tp4[:, :, 0:P])
            else:
                nc.vector.tensor_copy(dst[:, :, dst_col : dst_col + P], tp4[:, :, 0:P])

        # ---- All input loads upfront as quarter-sized DMAs ([P, 4, D] each):
        # few dispatcher slots, and the queues stay free of long-wait entries
        # so AllGather staging + reloads dispatch early.
        x2s = {}
        for t in range(MT):
            x2 = loads2.tile([P, D], F32, tag="x2", name="x2")
            nc.sync.dma_start(out=x2, in_=f2o[t * P : (t + 1) * P, :])
            x2s[t] = x2
        f1r = f1s[:, :].rearrange("(q i p) d -> q p i d", q=NG, i=GT, p=P)
        x1q = {}
        for q in range(NG):
            x1 = loads1.tile([P, GT, D], F32, tag="x1q", name="x1q")
            nc.sync.dma_start(out=x1, in_=f1r[q])
            x1q[q] = x1

        # ---- Prep (chunk-pipelined): f2 normalize-cast-transpose + AllGather
        # per quarter, then f1 cast-transpose + diag for the same quarter.
        for g in range(NG):
            gsl = slice(g * GT, (g + 1) * GT)
            for t in range(g * GT, (g + 1) * GT):
                sumsq_col(x2s[t], ss2g[:, t : t + 1], "2")
            ln2 = stats.tile([P, GT], F32, tag="ln", name="ln2")
            nc.scalar.activation(ln2, ss2g[:, gsl], AF.Ln)
            nc.scalar.activation(inv2g[:, gsl], ln2, AF.Exp, scale=-0.5)
            nc.vector.tensor_scalar_mul(inv2s[:, gsl], inv2g[:, gsl], SC)
            for t in range(g * GT, (g + 1) * GT):
                xq2 = xq.tile([P, D], BF16, tag="xq", name="xq2")
                nc.vector.tensor_scalar_mul(xq2, x2s.pop(t), inv2s[:, t : t + 1])
                transpose_pack(xq2, f2TlocG[g], (t - g * GT) * P, nc.scalar)
            nc.sync.dma_start(out=agin[g][:, :, :], in_=f2TlocG[g][:, :, :])
            nc.gpsimd.collective_compute(
                "AllGather",
                ALU.bypass,
                replica_groups=[list(range(N_CORES))],
                ins=[agin[g][:, :, :].opt()],
                outs=[agout[g][:, :, :, :].opt()],
            )
            # f1 prep for this quarter (no DMA here; tiles preloaded).
            for t in range(g * GT, (g + 1) * GT):
                x1 = x1q[g][:, t - g * GT, :]
                sumsq_col(x1, ss1g[:, t : t + 1], "1")
                xq1 = xq.tile([P, D], BF16, tag="xq", name="xq1")
                nc.vector.tensor_scalar_mul(xq1, x1, SC)
                transpose_pack(xq1, f1Tt[t], 0, nc.vector)
                # diag_sc2[:, t] = SC^2 * inv2 * (f1_row . f2_row) via PE.
                dps = psum.tile([P, P], F32, tag="ps", name="dps")
                for c in range(DC):
                    nc.tensor.matmul(
                        dps,
                        lhsT=f1Tt[t][:, c, :],
                        rhs=f2TlocG[g][:, c, (t - g * GT) * P : (t - g * GT + 1) * P],
                        start=(c == 0),
                        stop=(c == DC - 1),
                    )
                dmask = stats.tile([P, P], F32, tag="dm", name="dmask")
                nc.vector.tensor_tensor(dmask, dps, identity, ALU.mult)
                nc.vector.reduce_sum(
                    diag_sc2[:, t : t + 1], dmask, axis=mybir.AxisListType.X
                )
            x1q.pop(g)
            # Quarter scales: inv1 and the fused exp scale inv1/(SC^2*T).
            ln1 = stats.tile([P, GT], F32, tag="ln1", name="ln1")
            nc.scalar.activation(ln1, ss1g[:, gsl], AF.Ln)
            nc.scalar.activation(inv1g[:, gsl], ln1, AF.Exp, scale=-0.5)
            nc.vector.tensor_scalar_mul(scale1g[:, gsl], inv1g[:, gsl], eff_scale)

        # Reloads last in the DMA queues: unpack each chunk (all ranks, one
        # 4D DMA) into its SBUF-resident chunk tile.
        for g in range(NG):
            for r in range(N_CORES):
                nc.sync.dma_start(
                    out=f2Tg[g][:, :, r * GW : (r + 1) * GW],
                    in_=agout[g][r, :, :, :],
                )

        def mm_unit(ps_col, mt, g, rh):
            """8 DoubleRow matmuls accumulating [128, 2048] logits into psum:
            chunk g's strips for ranks 4rh..4rh+3, x 2 K-pairs."""
            for a in range(KP):
                for s in range(4):
                    col = (4 * rh + s) * GW
                    nc.tensor.matmul(
                        ps_col[:, s * GW : (s + 1) * GW],
                        lhsT=f1Tt[mt][:, 2 * a : 2 * a + 2, :],
                        rhs=f2Tg[g][:, 2 * a : 2 * a + 2, col : col + GW],
                        start=(a == 0),
                        stop=(a == KP - 1),
                        perf_mode=DR,
                    )

        # ---- Phase C0: chunk-0 strips, Exp straight from PSUM.
        for mt in range(MT):
            for rh in range(2):
                ps = psum.tile([P, 4 * GW], F32, tag="ps", name="ps0")
                mm_unit(ps, mt, 0, rh)
                nc.scalar.activation(
                    gexA[:, rh * 4 * GW : (rh + 1) * 4 * GW],
                    ps,
                    AF.Exp,
                    scale=scale1g[:, mt : mt + 1],
                    accum_out=rs[:, 4 * mt + rh : 4 * mt + rh + 1],
                )

        # Diag term for the loss, batched (consumed late, in phase W).
        nc.vector.tensor_tensor(diagterm, diag_sc2, scale1g, ALU.mult)

        # ---- Phase W: chunks 1-3, cast to SBUF, one wide Exp per m-tile
        # (two half-width Exps for the last m-tile to shorten the tail).
        for mt in range(MT):
            lg = logits.tile([P, WIDE], BF16, tag="lg", name="lg")
            for g in range(1, NG):
                for rh in range(2):
                    ps = psum.tile([P, 4 * GW], F32, tag="ps", name="psw")
                    mm_unit(ps, mt, g, rh)
                    off = ((g - 1) * 2 + rh) * 4 * GW
                    nc.vector.tensor_copy(lg[:, off : off + 4 * GW], ps)
                    if mt >= MT - 2 and g == 2 and rh == 1:
                        nc.scalar.activation(
                            gexW[:, 0 : 4 * 4 * GW],
                            lg[:, 0 : 4 * 4 * GW],
                            AF.Exp,
                            scale=scale1g[:, mt : mt + 1],
                            accum_out=rs[:, 4 * mt + 3 : 4 * mt + 4],
                        )
            if mt >= MT - 2:
                nc.scalar.activation(
                    gexW[:, 4 * 4 * GW : WIDE],
                    lg[:, 4 * 4 * GW : WIDE],
                    AF.Exp,
                    scale=scale1g[:, mt : mt + 1],
                    accum_out=rs[:, 4 * mt + 2 : 4 * mt + 3],
                )
            else:
                nc.scalar.activation(
                    gexW,
                    lg,
                    AF.Exp,
                    scale=scale1g[:, mt : mt + 1],
                    accum_out=rs[:, 4 * mt + 2 : 4 * mt + 3],
                )
            # logsumexp and loss column for this m-tile.
            s = stats.tile([P, 1], F32, tag="s", name="s")
            nc.vector.reduce_sum(
                s, rs[:, 4 * mt : 4 * mt + 4], axis=mybir.AxisListType.X
            )
            lse = stats.tile([P, 1], F32, tag="lse", name="lse")
            nc.scalar.activation(lse, s, AF.Ln)
            nc.vector.tensor_tensor(
                losses[:, mt : mt + 1], lse, diagterm[:, mt : mt + 1], ALU.subtract
            )

        # ---- Final reduction.
        loss_col = stats.tile([P, 1], F32, tag="lc", name="loss_col")
        nc.vector.reduce_sum(loss_col, losses, axis=mybir.AxisListType.X)
        fin = psum.tile([1, 1], F32, tag="ps", name="fin")
        nc.tensor.matmul(fin, lhsT=loss_col, rhs=ones_col, start=True, stop=True)
        res = stats.tile([1, 1], F32, tag="res", name="res")
        nc.any.tensor_copy(res, fin)
        nc.sync.dma_start(out=out[:, :], in_=res)

    return nc


_WAIT_EXEMPT = ("InstCall",)


def _legalize_sync_waits(nc, limit=1):
    """Walrus codegen rejects instructions carrying more than ~1 embedded
    semaphore wait ("Too many sync wait commands"). Move excess waits onto
    injected same-engine NoOps (one wait each) ahead of the instruction —
    semantically identical (the engine blocks on the NoOps first)."""
    n_split = 0
    for b in nc.m.functions[0].blocks:
        insts = b.instructions
        out = []
        changed = False
        for ins in insts:
            si = ins.sync_info
            tname = type(ins).__name__
            if (
                si is not None
                and len(si.on_wait) > limit
                and tname not in _WAIT_EXEMPT
            ):
                waits = list(si.on_wait)
                keep, excess = waits[:limit], waits[limit:]
                for j, w in enumerate(excess):
                    noop = mybir.InstNoOp(name=f"{ins.name}-ws{j}", ins=[], outs=[])
                    noop.engine = ins.engine
                    noop.sync_info = mybir.SyncInfo(on_wait=[w], on_update=[])
                    out.append(noop)
                ins.sync_info = mybir.SyncInfo(
                    on_wait=keep, on_update=list(si.on_update)
                )
                n_split += 1
                changed = True
            out.append(ins)
        if changed:
            b.instructions = out
    return n_split


def _maybe_patch_ldw_opt():
    """KERNEL_LDW_OPT=1 flips walrus --enable-ldw-opt to true (FWL weight
    loads); A/B experiment, correctness-checked by the rel-err gate."""
    if not int(os.environ.get("KERNEL_LDW_OPT", "0")):
        return
    import concourse.bass_utils as bu

    if getattr(bu.run_command, "_ldw_patched", False):
        return
    orig = bu.run_command

    def run2(cmd, **kw):
        cmd = [
            "--enable-ldw-opt=true" if c == "--enable-ldw-opt=false" else c
            for c in cmd
        ]
        return orig(cmd, **kw)

    run2._ldw_patched = True
    bu.run_command = run2


@lru_cache(maxsize=1)
def _get_nc():
    _maybe_patch_ldw_opt()
    nc = _build_bass()
    _legalize_sync_waits(nc)
    return nc


def kernel(features1, features2):
    global LAST_EXEC_TIME_NS
    f1 = np.ascontiguousarray(np.asarray(features1, dtype=np.float32))
    f2 = np.ascontiguousarray(np.asarray(features2, dtype=np.float32))
    assert f1.shape == (N, D) and f2.shape == (N, D)

    in_maps = []
    for i in range(N_CORES):
        sl = slice(i * M_LOCAL, (i + 1) * M_LOCAL)
        in_maps.append(
            {
                "f1s": np.ascontiguousarray(f1[sl]),
                "f2o": np.ascontiguousarray(f2[sl]),
            }
        )

    nc = _get_nc()
    trace = bool(int(os.environ.get("KERNEL_TRACE", "0")))
    if trace:
        _install_ntff_hook()
    tmpdir = os.environ.get("KERNEL_TRACE_DIR") or None
    r = run_bass_kernel_spmd(
        nc, in_maps, list(range(N_CORES)), trace=trace, tmpdir=tmpdir
    )
    LAST_EXEC_TIME_NS = r.exec_time_ns

    total = sum(float(r.results[i]["out"][0, 0]) for i in range(N_CORES))
    return np.float32(total / N)


# revision 15
# speedup vs baseline: 1.0566x; 1.0566x over previous
"""Distributed contrastive loss kernel for 8 Trainium2 NeuronCores.

loss = mean_i( logsumexp_j(f1n_i . f2n_j / T) - (f1n_i . f2n_i) / T )
with f1n/f2n the L2-row-normalized feature matrices, N=16384, D=512.

v2: fp8 DoubleRow matmuls + chunk-chasing AllGather + wide activations.

- f1 rows sharded 8 ways (2048/core); f2 rows likewise.
- Each core scale-casts its shards to fp8e4 (f1*4; f2*4*inv2 folding the
  row normalization into the quantization scale) and PE-transposes them
  into D-major layout [128, 4, cols].  The local f2 shard is AllGathered
  in 4 column-strip chunks (fp8 halves the wire bytes vs bf16).
- Main loop: per [128, 2048] PSUM tile, 8 DoubleRow fp8 matmuls (2 K-pairs
  x 4 rank-strips of 512 cols) accumulate f32 logits at 2 MACs/cell/cycle.
- Chunk-0 tiles are consumed by ScalarE Exp directly from PSUM (keeps PE
  fed while later AG chunks land); chunks 1-3 are DVE-cast to bf16 SBUF
  and consumed by one wide [128, 12288] Exp per m-tile, all with
  per-partition scale = inv1/(16*T) and accum_out row-sums.
- diag from fp8 PE matmuls of the local shards + identity-mask reduce;
  logsumexp via Ln of accumulated row sums; per-core partial reduced to a
  scalar with a ones-matmul.  Host sums 8 partials / N.
"""

import os
from contextlib import ExitStack
from functools import lru_cache

import numpy as np

import concourse.bass as bass
import concourse.mybir as mybir
import concourse.tile as tile
from concourse.bass_utils import run_bass_kernel_spmd
from concourse.masks import make_identity

# Problem shape (hardcoded per contest rules).
N = 16384
D = 512
N_CORES = 8
M_LOCAL = N // N_CORES  # 2048 rows per core
TEMP = 0.07

P = 128                 # SBUF partitions
DC = D // P             # 4 contraction chunks of 128
KP = DC // 2            # 2 DoubleRow K-pairs (each contracts 256)
MT = M_LOCAL // P       # 16 m-tiles per core
NG = 4                  # AllGather chunks (column strips of 512 per rank)
GT = MT // NG           # 4 f2-shard tiles per chunk
GW = GT * P             # 512 columns per chunk strip
SC = 4.0                # fp8 quantization pre-scale on both operands
F32 = mybir.dt.float32
BF16 = mybir.dt.bfloat16
FP8 = mybir.dt.float8e4
AF = mybir.ActivationFunctionType
ALU = mybir.AluOpType
DR = mybir.MatmulPerfMode.DoubleRow

# Module-level stash for the last run's profile (read by test.py).
LAST_EXEC_TIME_NS = None


def _install_ntff_hook():
    """Provide antenv.axon_hooks (missing from this image) so that
    run_bass_kernel_spmd(trace=True) can capture NTFF profiles via the
    axon PJRT .so. Mirrors trn_agent_boot.trn_boot._ntff_profile_via_ctypes."""
    import contextlib
    import ctypes
    import sys
    import types

    try:
        import antenv.axon_hooks  # noqa: F401

        return
    except ImportError:
        pass

    so_path = "/opt/axon/libaxon_pjrt.so"
    hook = None
    try:
        lib = ctypes.CDLL(so_path)
        if hasattr(lib, "axon_start_nrt_profile"):
            lib.axon_start_nrt_profile.argtypes = [
                ctypes.POINTER(ctypes.c_int64),
                ctypes.c_size_t,
            ]
            lib.axon_start_nrt_profile.restype = ctypes.c_int64
            lib.axon_stop_nrt_profile.argtypes = [ctypes.c_char_p]
            lib.axon_stop_nrt_profile.restype = ctypes.c_int64

            @contextlib.contextmanager
            def _hook(output_dir, device_ids):
                import jax

                jax.devices()
                if device_ids:
                    ids = (ctypes.c_int64 * len(device_ids))(*device_ids)
                    rc = lib.axon_start_nrt_profile(ids, len(device_ids))
                else:
                    rc = lib.axon_start_nrt_profile(None, 0)
                if rc != 0:
                    raise RuntimeError(f"axon_start_nrt_profile rc={rc}")
                try:
                    yield
                finally:
                    n = lib.axon_stop_nrt_profile(str(output_dir).encode())
                    print(f"profile: {n} file(s) written to {output_dir}", file=sys.stderr)

            hook = _hook
    except OSError:
        pass

    import antenv

    mod = types.ModuleType("antenv.axon_hooks")
    _state = {"hook": hook}
    mod.set_axon_ntff_profile_hook = lambda h: _state.__setitem__("hook", h)
    mod.get_axon_ntff_profile_hook = lambda: _state["hook"]
    sys.modules["antenv.axon_hooks"] = mod
    antenv.axon_hooks = mod

    # Artifact upload needs external storage creds; make it a no-op.
    import concourse.bass_utils as _bu

    _bu.upload_artifacts = lambda tmpdir: f"local:{tmpdir}"


def _build_bass():
    nc = bass.Bass(num_devices=N_CORES, debug=False)

    f1s = nc.dram_tensor("f1s", [M_LOCAL, D], F32, kind="ExternalInput")
    f2o = nc.dram_tensor("f2o", [M_LOCAL, D], F32, kind="ExternalInput")
    out = nc.dram_tensor("out", [1, 1], F32, kind="ExternalOutput")

    # Exp scale: psum holds SC^2 * (f1 . f2n); true logit needs * inv1 / T.
    eff_scale = 1.0 / (SC * SC * TEMP)
    WIDE = (NG - 1) * N_CORES * GW  # 12288 cols handled by the wide path

    with tile.TileContext(nc) as tc, ExitStack() as ctx:
        consts = ctx.enter_context(tc.tile_pool(name="consts", bufs=1))
        resident = ctx.enter_context(tc.tile_pool(name="resident", bufs=1))
        loads1 = ctx.enter_context(tc.tile_pool(name="loads1", bufs=4))
        loads2 = ctx.enter_context(tc.tile_pool(name="loads2", bufs=3))
        xq = ctx.enter_context(tc.tile_pool(name="xq", bufs=3))
        stats = ctx.enter_context(tc.tile_pool(name="stats", bufs=4))
        logits = ctx.enter_context(tc.tile_pool(name="logits", bufs=2))
        psum = ctx.enter_context(tc.tile_pool(name="psum", bufs=2, space="PSUM"))
        dram = ctx.enter_context(tc.tile_pool(name="dram", bufs=1, space="DRAM"))

        identity = consts.tile([P, P], BF16)
        make_identity(nc, identity)
        ones_col = consts.tile([P, 1], F32)
        nc.vector.memset(ones_col, 1.0)

        # Residents.  Split per-mtile / per-chunk so the byte-interval dep
        # tracker never sees false write/read overlaps across chunks.
        f1Tt = [
            resident.tile([P, DC, P], FP8, name=f"f1T{t}") for t in range(MT)
        ]
        f2TlocG = [
            resident.tile([P, DC, GW], FP8, name=f"f2Tloc{g}") for g in range(NG)
        ]
        f2Tg = [
            resident.tile([P, DC, N_CORES * GW], FP8, name=f"f2T{g}")
            for g in range(NG)
        ]
        ss1g = resident.tile([P, MT], F32, name="ss1g")
        ss2g = resident.tile([P, MT], F32, name="ss2g")
        inv1g = resident.tile([P, MT], F32, name="inv1g")
        inv2g = resident.tile([P, MT], F32, name="inv2g")
        inv2s = resident.tile([P, MT], F32, name="inv2s")
        scale1g = resident.tile([P, MT], F32, name="scale1g")
        diag_sc2 = resident.tile([P, MT], F32, name="diag_sc2")
        diagterm = resident.tile([P, MT], F32, name="diagterm")
        rs = resident.tile([P, 4 * MT], F32, name="rs")
        losses = resident.tile([P, MT], F32, name="losses")
        gexW = resident.tile([P, WIDE], FP8, name="gexW")
        nc.vector.memset(rs, 0.0)

        # AllGather bounce buffers, one pair per chunk.
        agin = [
            dram.tile([P, DC, GW], FP8, name=f"agin{g}") for g in range(NG)
        ]
        agout = [
            dram.tile([N_CORES, P, DC, GW], FP8, name=f"agout{g}",
                      addr_space="Shared")
            for g in range(NG)
        ]

        def sumsq_col(x, col, tag):
            """sum(x^2) per row of [P, D] f32 tile -> col ([P,1] slice). DVE only."""
            st = stats.tile([P, nc.vector.BN_STATS_DIM], F32, tag="bst", name=f"st{tag}")
            nc.vector.bn_stats(st, x)
            mv = stats.tile([P, nc.vector.BN_AGGR_DIM], F32, tag="bmv", name=f"mv{tag}")
            nc.vector.bn_aggr(mv, st)
            m2 = stats.tile([P, 1], F32, tag="m2", name=f"m2{tag}")
            nc.vector.tensor_tensor(m2, mv[:, 0:1], mv[:, 0:1], ALU.mult)
            s2 = stats.tile([P, 1], F32, tag="s2", name=f"s2{tag}")
            nc.vector.tensor_tensor(s2, mv[:, 1:2], m2, ALU.add)
            nc.vector.tensor_scalar_mul(col, s2, float(D))

        def transpose_pack(xq_tile, dst, dst_col, eng):
            """4x bf16 PE transpose of [P, D] into D-major dst[:, :, dst_col:+P].
            Each 128x128 transpose lands in its own PSUM bank; the fp8
            quantization happens in the PSUM->SBUF copy on `eng`."""
            tp4 = psum.tile([P, DC, 1024], BF16, tag="ps", name="tp4")
            for c in range(DC):
                nc.tensor.matmul(
                    tp4[:, c, 0:P],
                    lhsT=xq_tile[:, c * P : (c + 1) * P],
                    rhs=identity,
                    is_transpose=True,
                )
            if eng is nc.scalar:
                nc.scalar.copy(dst[:, :, dst_col : dst_col + P], tp4[:, :, 0:P])
            else:
                nc.vector.tensor_copy(dst[:, :, dst_col : dst_col + P], tp4[:, :, 0:P])

        # ---- All input loads upfront as quarter-sized DMAs ([P, 4, D] each):
        # few dispatcher slots, and the queues stay free of long-wait entries
        # so AllGather staging + reloads dispatch early.
        f2r = f2o[:, :].rearrange("(q i p) d -> q p i d", q=NG, i=GT, p=P)
        x2q = {}
        for q in range(NG):
            x2 = loads2.tile([P, GT, D], F32, tag="x2q", name="x2q")
            nc.sync.dma_start(out=x2, in_=f2r[q])
            x2q[q] = x2
        f1r = f1s[:, :].rearrange("(q i p) d -> q p i d", q=NG, i=GT, p=P)
        x1q = {}
        for q in range(NG):
            x1 = loads1.tile([P, GT, D], F32, tag="x1q", name="x1q")
            nc.sync.dma_start(out=x1, in_=f1r[q])
            x1q[q] = x1

        # ---- Prep (chunk-pipelined): f2 normalize-cast-transpose + AllGather
        # per quarter, then f1 cast-transpose + diag for the same quarter.
        for g in range(NG):
            gsl = slice(g * GT, (g + 1) * GT)
            for t in range(g * GT, (g + 1) * GT):
                sumsq_col(x2q[g][:, t - g * GT, :], ss2g[:, t : t + 1], "2")
            ln2 = stats.tile([P, GT], F32, tag="ln", name="ln2")
            nc.scalar.activation(ln2, ss2g[:, gsl], AF.Ln)
            nc.scalar.activation(inv2g[:, gsl], ln2, AF.Exp, scale=-0.5)
            nc.vector.tensor_scalar_mul(inv2s[:, gsl], inv2g[:, gsl], SC)
            for t in range(g * GT, (g + 1) * GT):
                xq2 = xq.tile([P, D], BF16, tag="xq", name="xq2")
                nc.vector.tensor_scalar_mul(xq2, x2q[g][:, t - g * GT, :], inv2s[:, t : t + 1])
                transpose_pack(xq2, f2TlocG[g], (t - g * GT) * P, nc.scalar)
            x2q.pop(g)
            nc.sync.dma_start(out=agin[g][:, :, :], in_=f2TlocG[g][:, :, :])
            nc.gpsimd.collective_compute(
                "AllGather",
                ALU.bypass,
                replica_groups=[list(range(N_CORES))],
                ins=[agin[g][:, :, :].opt()],
                outs=[agout[g][:, :, :, :].opt()],
            )
            # f1 prep for this quarter (no DMA here; tiles preloaded).
            for t in range(g * GT, (g + 1) * GT):
                x1 = x1q[g][:, t - g * GT, :]
                sumsq_col(x1, ss1g[:, t : t + 1], "1")
                xq1 = xq.tile([P, D], BF16, tag="xq", name="xq1")
                nc.vector.tensor_scalar_mul(xq1, x1, SC)
                transpose_pack(xq1, f1Tt[t], 0, nc.vector)
                # diag_sc2[:, t] = SC^2 * inv2 * (f1_row . f2_row) via PE.
                dps = psum.tile([P, P], F32, tag="ps", name="dps")
                for c in range(DC):
                    nc.tensor.matmul(
                        dps,
                        lhsT=f1Tt[t][:, c, :],
                        rhs=f2TlocG[g][:, c, (t - g * GT) * P : (t - g * GT + 1) * P],
                        start=(c == 0),
                        stop=(c == DC - 1),
                    )
                dmask = stats.tile([P, P], F32, tag="dm", name="dmask")
                nc.vector.tensor_tensor(dmask, dps, identity, ALU.mult)
                nc.vector.reduce_sum(
                    diag_sc2[:, t : t + 1], dmask, axis=mybir.AxisListType.X
                )
            x1q.pop(g)
            # Quarter scales: inv1 and the fused exp scale inv1/(SC^2*T).
            ln1 = stats.tile([P, GT], F32, tag="ln1", name="ln1")
            nc.scalar.activation(ln1, ss1g[:, gsl], AF.Ln)
            nc.scalar.activation(inv1g[:, gsl], ln1, AF.Exp, scale=-0.5)
            nc.vector.tensor_scalar_mul(scale1g[:, gsl], inv1g[:, gsl], eff_scale)

        # Reloads last in the DMA queues: unpack each chunk (all ranks, one
        # 4D DMA) into its SBUF-resident chunk tile.
        for g in range(NG):
            for r in range(N_CORES):
                nc.sync.dma_start(
                    out=f2Tg[g][:, :, r * GW : (r + 1) * GW],
                    in_=agout[g][r, :, :, :],
                )

        def mm_unit(ps_col, mt, g, rh):
            """8 DoubleRow matmuls accumulating [128, 2048] logits into psum:
            chunk g's strips for ranks 4rh..4rh+3, x 2 K-pairs."""
            for a in range(KP):
                for s in range(4):
                    col = (4 * rh + s) * GW
                    nc.tensor.matmul(
                        ps_col[:, s * GW : (s + 1) * GW],
                        lhsT=f1Tt[mt][:, 2 * a : 2 * a + 2, :],
                        rhs=f2Tg[g][:, 2 * a : 2 * a + 2, col : col + GW],
                        start=(a == 0),
                        stop=(a == KP - 1),
                        perf_mode=DR,
                    )

        # ---- Phase C0: chunk-0 strips, Exp straight from PSUM.
        for mt in range(MT):
            for rh in range(2):
                ps = psum.tile([P, 4 * GW], F32, tag="ps", name="ps0")
                mm_unit(ps, mt, 0, rh)
                nc.scalar.activation(
                    gexW[:, rh * 4 * GW : (rh + 1) * 4 * GW],
                    ps,
                    AF.Exp,
                    scale=scale1g[:, mt : mt + 1],
                    accum_out=rs[:, 4 * mt + rh : 4 * mt + rh + 1],
                )

        # Diag term for the loss, batched (consumed late, in phase W).
        nc.vector.tensor_tensor(diagterm, diag_sc2, scale1g, ALU.mult)

        # ---- Phase W: chunks 1-3, cast to SBUF, one wide Exp per m-tile
        # (two half-width Exps for the last m-tile to shorten the tail).
        for mt in range(MT):
            lg = logits.tile([P, WIDE], BF16, tag="lg", name="lg")
            for g in range(1, NG):
                for rh in range(2):
                    ps = psum.tile([P, 4 * GW], F32, tag="ps", name="psw")
                    mm_unit(ps, mt, g, rh)
                    off = ((g - 1) * 2 + rh) * 4 * GW
                    nc.vector.tensor_copy(lg[:, off : off + 4 * GW], ps)
                    if mt >= MT - 2 and g == 2 and rh == 1:
                        nc.scalar.activation(
                            gexW[:, 0 : 4 * 4 * GW],
                            lg[:, 0 : 4 * 4 * GW],
                            AF.Exp,
                            scale=scale1g[:, mt : mt + 1],
                            accum_out=rs[:, 4 * mt + 3 : 4 * mt + 4],
                        )
            if mt >= MT - 2:
                nc.scalar.activation(
                    gexW[:, 4 * 4 * GW : WIDE],
                    lg[:, 4 * 4 * GW : WIDE],
                    AF.Exp,
                    scale=scale1g[:, mt : mt + 1],
                    accum_out=rs[:, 4 * mt + 2 : 4 * mt + 3],
                )
            else:
                nc.scalar.activation(
                    gexW,
                    lg,
                    AF.Exp,
                    scale=scale1g[:, mt : mt + 1],
                    accum_out=rs[:, 4 * mt + 2 : 4 * mt + 3],
                )
            # logsumexp and loss column for this m-tile.
            s = stats.tile([P, 1], F32, tag="s", name="s")
            nc.vector.reduce_sum(
                s, rs[:, 4 * mt : 4 * mt + 4], axis=mybir.AxisListType.X
            )
            lse = stats.tile([P, 1], F32, tag="lse", name="lse")
            nc.scalar.activation(lse, s, AF.Ln)
            nc.vector.tensor_tensor(
                losses[:, mt : mt + 1], lse, diagterm[:, mt : mt + 1], ALU.subtract
            )

        # ---- Final reduction.
        loss_col = stats.tile([P, 1], F32, tag="lc", name="loss_col")
        nc.vector.reduce_sum(loss_col, losses, axis=mybir.AxisListType.X)
        fin = psum.tile([1, 1], F32, tag="ps", name="fin")
        nc.tensor.matmul(fin, lhsT=loss_col, rhs=ones_col, start=True, stop=True)
        res = stats.tile([1, 1], F32, tag="res", name="res")
        nc.any.tensor_copy(res, fin)
        nc.sync.dma_start(out=out[:, :], in_=res)

    return nc


_WAIT_EXEMPT = ("InstCall",)


def _legalize_sync_waits(nc, limit=1):
    """Walrus codegen rejects instructions carrying more than ~1 embedded
    semaphore wait ("Too many sync wait commands"). Move excess waits onto
    injected same-engine NoOps (one wait each) ahead of the instruction —
    semantically identical (the engine blocks on the NoOps first)."""
    n_split = 0
    for b in nc.m.functions[0].blocks:
        insts = b.instructions
        out = []
        changed = False
        for ins in insts:
            si = ins.sync_info
            tname = type(ins).__name__
            if (
                si is not None
                and len(si.on_wait) > limit
                and tname not in _WAIT_EXEMPT
            ):
                waits = list(si.on_wait)
                keep, excess = waits[:limit], waits[limit:]
                for j, w in enumerate(excess):
                    noop = mybir.InstNoOp(name=f"{ins.name}-ws{j}", ins=[], outs=[])
                    noop.engine = ins.engine
                    noop.sync_info = mybir.SyncInfo(on_wait=[w], on_update=[])
                    out.append(noop)
                ins.sync_info = mybir.SyncInfo(
                    on_wait=keep, on_update=list(si.on_update)
                )
                n_split += 1
                changed = True
            out.append(ins)
        if changed:
            b.instructions = out
    return n_split


def _maybe_patch_ldw_opt():
    """KERNEL_LDW_OPT=1 flips walrus --enable-ldw-opt to true (FWL weight
    loads); A/B experiment, correctness-checked by the rel-err gate."""
    if not int(os.environ.get("KERNEL_LDW_OPT", "0")):
        return
    import concourse.bass_utils as bu

    if getattr(bu.run_command, "_ldw_patched", False):
        return
    orig = bu.run_command

    def run2(cmd, **kw):
        cmd = [
            "--enable-ldw-opt=true" if c == "--enable-ldw-opt=false" else c
            for c in cmd
        ]
        return orig(cmd, **kw)

    run2._ldw_patched = True
    bu.run_command = run2


@lru_cache(maxsize=1)
def _get_nc():
    _maybe_patch_ldw_opt()
    nc = _build_bass()
    _legalize_sync_waits(nc)
    return nc


def kernel(features1, features2):
    global LAST_EXEC_TIME_NS
    f1 = np.ascontiguousarray(np.asarray(features1, dtype=np.float32))
    f2 = np.ascontiguousarray(np.asarray(features2, dtype=np.float32))
    assert f1.shape == (N, D) and f2.shape == (N, D)

    in_maps = []
    for i in range(N_CORES):
        sl = slice(i * M_LOCAL, (i + 1) * M_LOCAL)
        in_maps.append(
            {
                "f1s": np.ascontiguousarray(f1[sl]),
                "f2o": np.ascontiguousarray(f2[sl]),
            }
        )

    nc = _get_nc()
    trace = bool(int(os.environ.get("KERNEL_TRACE", "0")))
    if trace:
        _install_ntff_hook()
    tmpdir = os.environ.get("KERNEL_TRACE_DIR") or None
    r = run_bass_kernel_spmd(
        nc, in_maps, list(range(N_CORES)), trace=trace, tmpdir=tmpdir
    )
    LAST_EXEC_TIME_NS = r.exec_time_ns

    total = sum(float(r.results[i]["out"][0, 0]) for i in range(N_CORES))
    return np.float32(total / N)
